# revision 18
# baseline (speedup 1.0000x reference)
"""Trainium2 Bass kernel for a 4-layer stacked LSTM (hidden=25, in=1) + FC head.

Problem: x [512, 4096, 1] -> scan over 512 steps of 4 LSTMCells + Linear(25,1).
Returns (y [512,4096,1], (h1,c1,h2,c2,h3,c3,h4,c4) each [4096,25]).

Strategy (per NeuronCore, batch 4096 sharded 8 ways -> 512/core, further split
into 2 streams of 256 to hide recurrence latency):

- Wavefront pipeline over slots s: layer l (1-based) processes t = s-(l-1).
  All four layers' matmuls within a slot only read state from slot s-1, so
  they are independent.
- State tile H [128, B]: rows 25l..25l+24 = h'_{l+1} (h scaled by 2), row 100
  = x_t, row 101 = ones (bias lane). C tile: rows 25l.. = c' = 2c.
- One matmul per *gate type* (i,f,o,g) with a block-banded stationary
  [K=102, M=100] covering all 4 layers at once -> gates land in PSUM already
  partition-aligned with the states (layer-major rows). The FC head rides the
  g-gate matmul as one extra stationary column (M=101); row 100 of the G bank
  is y_t and is DMA'd straight from PSUM to DRAM.
- All gate nonlinearities in ONE tanh activation per slot over the 4 gate
  banks [100, 4B]: sigmoid(z) = 0.5*(1+tanh(z/2)) handled by scale=0.5 with
  g-gate weight columns pre-doubled; the 0.5/+1 fixups fold into
  scalar_tensor_tensor ops by storing h'=2h, c'=2c (weights on h-rows are
  pre-halved on the host):
      A  = (T_f + 1) * c'_prev            (DVE)
      B  = (T_i + 1) * T_g                (GPSIMD)
      c' = (A * 0.5) + B                  (DVE)
      TC = tanh(0.5 * c')                 (ACT)
      h' = (T_o + 1) * TC                 (DVE)
- Matmuls run in float32r (full-rate PE streaming, fp32 bits).
"""

import sys

import numpy as np

if "/opt/trn_rl_repo" not in sys.path:
    sys.path.insert(0, "/opt/trn_rl_repo")

SIZE = 25
LEN = 512
BSIZE = 4096
N_CORES = 8
CORE_B = BSIZE // N_CORES  # 512
N_STREAMS = 2
SB = CORE_B // N_STREAMS  # 256
# state rows 0..99 = h'(4 layers x 25); row 100 = x_t; rows 101..104 = per-layer
# phase-in "ones" rows (0 until the layer's first slot, then 1) carrying biases
KDIM = 105
M_G = 100  # gate columns (4 layers x 25)
X_ROW = 100
Y_ROW = 100  # row of the G psum/T chunk holding y


def build_nc(seq_len=LEN):
    import concourse.bacc as bacc
    import concourse.mybir as mybir
    from concourse.tile import TileContext

    f32 = mybir.dt.float32
    f32r = mybir.dt.float32r
    bf16 = mybir.dt.bfloat16
    ADD = mybir.AluOpType.add
    MULT = mybir.AluOpType.mult
    TANH = mybir.ActivationFunctionType.Tanh

    nc = bacc.Bacc()

    # xext rows per step: [x_t, m1, m2, m3, m4] (phase-in masks)
    x_dr = nc.dram_tensor("x", [seq_len, 5, CORE_B], f32r, kind="ExternalInput")
    # packed stationaries: gate g in cols [g*128, g*128+101]
    w_dr = nc.dram_tensor("w", [128, 512], f32r, kind="ExternalInput")
    y_dr = nc.dram_tensor("y", [seq_len, CORE_B], f32, kind="ExternalOutput")
    # h'1..h'4, c'1..c'4 (doubled scale)
    hc_dr = nc.dram_tensor("hc", [8, SIZE, CORE_B], f32, kind="ExternalOutput")

    n_slots = seq_len + 4  # slots 0 .. seq_len+3; last slot runs only mm_G for y

    with TileContext(nc) as tc:
        with (
            tc.tile_pool(name="main", bufs=1) as pool,
            tc.tile_pool(name="ps", bufs=1, space="PSUM") as pspool,
        ):
            w_sb = pool.tile([128, 512], f32r)
            nc.sync.dma_start(w_sb[:, :], w_dr[:, :], single_packet=True)

            H = [[pool.tile([128, SB], f32r, name=f"H_{a}_{p}") for p in (0, 1)]
                 for a in range(N_STREAMS)]
            C = [[pool.tile([128, SB], f32, name=f"C_{a}_{p}") for p in (0, 1)]
                 for a in range(N_STREAMS)]
            T = [[pool.tile([128, 4 * SB], f32, name=f"T_{a}_{p}") for p in (0, 1)]
                 for a in range(N_STREAMS)]
            At = [pool.tile([128, SB], f32, name=f"A_{a}") for a in range(N_STREAMS)]
            Bt = [pool.tile([128, SB], f32, name=f"B_{a}") for a in range(N_STREAMS)]
            TC_t = [pool.tile([128, SB], f32, name=f"TC_{a}") for a in range(N_STREAMS)]
            PS = [[pspool.tile([128, 4 * SB], f32, name=f"PS_{a}_{p}") for p in (0, 1)]
                  for a in range(N_STREAMS)]

            # init: zero states, prologue xext(0), xext(1)
            for a in range(N_STREAMS):
                for p in (0, 1):
                    nc.vector.memset(H[a][p][0:100, :].bitcast(f32), 0.0)
                    nc.vector.memset(C[a][p][0:100, :], 0.0)
                    nc.sync.dma_start(
                        H[a][p][X_ROW:X_ROW + 5, :],
                        x_dr[p, :, a * SB:(a + 1) * SB],
                        single_packet=True,
                    )

            for s in range(n_slots):
                prv = s % 2
                cur = 1 - prv
                last = s == n_slots - 1
                for a in range(N_STREAMS):
                    ps_t = PS[a][prv]
                    T_t = T[a][prv]
                    rhs = H[a][prv][0:KDIM, :]
                    # matmuls: gate order in psum free dim: I, F, O, G
                    # M=101 for all gates (zero col for I/F/O; FC col for G)
                    for g in range(4):
                        if last and g != 3:
                            continue
                        nc.tensor.matmul(
                            ps_t[0:M_G + 1, g * SB:g * SB + SB],
                            w_sb[0:KDIM, g * 128:g * 128 + M_G + 1],
                            rhs,
                            start=True,
                            stop=True,
                        )
                    if last:
                        # only y(seq_len-1) remains: tanh the G chunk, DMA
                        nc.scalar.activation(
                            T_t[0:101, 3 * SB:4 * SB],
                            ps_t[0:101, 3 * SB:4 * SB], TANH, scale=0.5,
                        )
                        nc.sync.dma_start(
                            y_dr[s - 4:s - 3, a * SB:(a + 1) * SB],
                            T_t[Y_ROW:Y_ROW + 1, 3 * SB:4 * SB],
                        )
                        continue

                    # T = tanh(0.5 * gates)  [101, 4*SB]; row 100 of the G
                    # chunk is tanh(0.5*y) (undone host-side via atanh)
                    nc.scalar.activation(
                        T_t[0:101, :], ps_t[0:101, :], TANH, scale=0.5
                    )
                    # y(s-4)
                    if s >= 4:
                        nc.sync.dma_start(
                            y_dr[s - 4:s - 3, a * SB:(a + 1) * SB],
                            T_t[Y_ROW:Y_ROW + 1, 3 * SB:4 * SB],
                        )
                    t_i = T_t[0:100, 0 * SB:1 * SB]
                    t_f = T_t[0:100, 1 * SB:2 * SB]
                    t_o = T_t[0:100, 2 * SB:3 * SB]
                    t_g = T_t[0:100, 3 * SB:4 * SB]

                    # A = (T_f + 1) * c'_prev
                    nc.vector.scalar_tensor_tensor(
                        At[a][0:100, :], t_f, 1.0, C[a][prv][0:100, :], ADD, MULT
                    )
                    # B = (T_i + 1) * T_g
                    nc.vector.scalar_tensor_tensor(
                        Bt[a][0:100, :], t_i, 1.0, t_g, ADD, MULT
                    )
                    # c'_cur = A*0.5 + B
                    nc.vector.scalar_tensor_tensor(
                        C[a][cur][0:100, :], At[a][0:100, :], 0.5,
                        Bt[a][0:100, :], MULT, ADD,
                    )
                    # TC = tanh(0.5 * c')
                    nc.scalar.activation(
                        TC_t[a][0:100, :], C[a][cur][0:100, :], TANH, scale=0.5
                    )
                    # h'_cur = (T_o + 1) * TC
                    nc.vector.scalar_tensor_tensor(
                        H[a][cur][0:100, :], t_o, 1.0, TC_t[a][0:100, :], ADD, MULT
                    )

                    # prefetch xext(s+2) into H[prv] rows 100-104
                    if s + 2 <= seq_len - 1:
                        nc.gpsimd.dma_start(
                            H[a][prv][X_ROW:X_ROW + 5, :],
                            x_dr[s + 2, :, a * SB:(a + 1) * SB],
                        )

                    # final h/c for layer li finish at slot seq_len-1+li
                    li = s - (seq_len - 1)
                    if 0 <= li <= 3:
                        nc.gpsimd.dma_start(
                            hc_dr[li, :, a * SB:(a + 1) * SB],
                            H[a][cur][25 * li:25 * li + 25, :],
                        )
                        nc.gpsimd.dma_start(
                            hc_dr[4 + li, :, a * SB:(a + 1) * SB],
                            C[a][cur][25 * li:25 * li + 25, :],
                        )
    nc.compile()
    return nc


def pack_weights(inp):
    """Build the [128, 512] packed stationary from the problem's weights."""
    w = np.zeros((128, 512), dtype=np.float32)
    # torch LSTM gate order in the 4s rows: (i, f, g, o)
    torder = {"i": 0, "f": 1, "g": 2, "o": 3}
    gate_list = ["i", "f", "o", "g"]  # our psum free order
    for gidx, gate in enumerate(gate_list):
        tg = torder[gate]
        dbl = 2.0 if gate == "g" else 1.0
        for l in range(4):  # layer index 0-based
            W_ih = np.asarray(inp[f"W_ih{l + 1}"], dtype=np.float32)
            W_hh = np.asarray(inp[f"W_hh{l + 1}"], dtype=np.float32)
            b = (np.asarray(inp[f"b_ih{l + 1}"]) + np.asarray(inp[f"b_hh{l + 1}"])).astype(np.float32)
            rows = slice(tg * SIZE, (tg + 1) * SIZE)  # rows of torch weight mats
            cols = slice(gidx * 128 + 25 * l, gidx * 128 + 25 * l + 25)
            # hh part: input h'_l lives at partition rows 25l.. ; halve (h'=2h)
            w[25 * l:25 * l + 25, cols] += 0.5 * dbl * W_hh[rows, :].T
            if l == 0:
                # x row
                w[X_ROW, cols] = dbl * W_ih[rows, 0]
            else:
                # ih part: input h'_{l-1} at rows 25(l-1)..
                w[25 * (l - 1):25 * (l - 1) + 25, cols] += 0.5 * dbl * W_ih[rows, :].T
            # bias rides layer l's phase-in ones row
            w[101 + l, cols] = dbl * b[rows]
        if gate == "g":
            # FC head column (not doubled): y = 0.5*W_fc @ h'4 + b_fc
            W_fc = np.asarray(inp["W_fc"], dtype=np.float32)  # [1, 25]
            b_fc = np.asarray(inp["b_fc"], dtype=np.float32)  # [1]
            col = gidx * 128 + 100
            w[75:100, col] = 0.5 * W_fc[0, :]
            w[101 + 3, col] = b_fc[0]
    return w


def make_xext(x_core):
    """x_core [LEN, CORE_B] -> [LEN, 5, CORE_B]: x plus per-layer phase-in
    masks (slot s: layer l active iff s >= l-1)."""
    seq_len = x_core.shape[0]
    xext = np.zeros((seq_len, 5, x_core.shape[1]), dtype=np.float32)
    xext[:, 0, :] = x_core
    for l in range(4):
        xext[l:, 1 + l, :] = 1.0
    return xext


_NC_CACHE = {}
LAST_RESULT = None


def _get_nc(seq_len=LEN):
    if seq_len not in _NC_CACHE:
        _NC_CACHE[seq_len] = build_nc(seq_len)
    return _NC_CACHE[seq_len]


def kernel(**inputs):
    from concourse import bass_utils

    x = np.asarray(inputs["x"], dtype=np.float32)  # [512, 4096, 1]
    seq_len = x.shape[0]
    w_pack = pack_weights(inputs)

    nc = _get_nc(seq_len)
    in_maps = []
    for c in range(N_CORES):
        xs = x[:, c * CORE_B:(c + 1) * CORE_B, 0]  # [LEN, 512]
        in_maps.append({"x": make_xext(xs), "w": w_pack})

    global LAST_RESULT
    res = bass_utils.run_bass_kernel_spmd(nc, in_maps, core_ids=list(range(N_CORES)))
    LAST_RESULT = res

    y = np.empty((seq_len, BSIZE, 1), dtype=np.float32)
    hs = [np.empty((BSIZE, SIZE), dtype=np.float32) for _ in range(4)]
    cs = [np.empty((BSIZE, SIZE), dtype=np.float32) for _ in range(4)]
    for c in range(N_CORES):
        r = res.results[c]
        ty = np.clip(r["y"], -1.0 + 1e-7, 1.0 - 1e-7)
        y[:, c * CORE_B:(c + 1) * CORE_B, 0] = 2.0 * np.arctanh(ty)
        hc = r["hc"]  # [8, 25, CORE_B], doubled scale
        for l in range(4):
            hs[l][c * CORE_B:(c + 1) * CORE_B, :] = 0.5 * hc[l].T
            cs[l][c * CORE_B:(c + 1) * CORE_B, :] = 0.5 * hc[4 + l].T
    return (y, (hs[0], cs[0], hs[1], cs[1], hs[2], cs[2], hs[3], cs[3]))
